# revision 6
# baseline (speedup 1.0000x reference)
"""Causal self-attention (RoPE + QK-RMSNorm) Trainium2 kernel, 8-core tensor-parallel.

Problem: B=4, S=2048, E=2048, H=16 heads, D=128, fp32.
Sharding: tensor-parallel over heads -- each core computes 2 heads end-to-end
(QKV projection, RoPE, QK-norm, causal attention, output projection) and
returns a partial output [B*S, E]; the host sums the 8 partials.

Per-core data layouts (contraction dims on partitions):
  xT   [E, B*S]    input transposed on host (f32r)
  QT/KT [D, S]     per (b,h); rows host-permuted to [even dims | odd dims] so
                   RoPE pairs become partition halves (swap via one matmul
                   against a +-1 permutation matrix J)
  VT   [D, S] -> V [S, D] via PE transpose (ctx matmul needs V natural)
  scores^T [k, q]  softmax denominator via all-ones matmul (partition sums);
                   ctx^T = V.T @ exp(scores^T) -- no transposes in attention
  ctxT [D, S]      stationary operand of the output projection

Numerics: matmuls in f32r (~1.5e-4 rel err, full PE rate at N>=256); softmax
without max-subtraction (qk-norm bounds |score| <= sqrt(D): exp <= 9e4, no
overflow); reciprocal/rsqrt via ACT Exp(-a*Ln(x)) so only one ACT table set
(natural_log_exp) is ever loaded.
"""

import sys

sys.path.insert(0, "/opt/trn_rl_repo")

import numpy as np
from contextlib import ExitStack

import concourse.bass as bass
import concourse.mybir as mybir
import concourse.tile as tile
from concourse import bacc
from concourse.bass_utils import run_bass_kernel_spmd

F32 = mybir.dt.float32
F32R = mybir.dt.float32r
AF = mybir.ActivationFunctionType

N_CORES = 8
N_HEAD = 16
ROPE_BASE = 10000.0
QK_NORM_EPS = 1e-5

B, S, E = 4, 2048, 2048
D = E // N_HEAD          # 128
HPC = N_HEAD // N_CORES  # heads per core


def build_kernel(b_=B, s_=S):
    """Build the per-core Bass program for batch size b_ and seqlen s_."""
    ROWS = b_ * s_
    QB = min(512, s_)     # q-block width in attention
    NQB = s_ // QB
    KPQ = QB // 128       # k-tiles spanned by one q-block (diag band width)
    NE = E // 128
    RC = min(512, s_)     # phase-A row chunk

    nc = bacc.Bacc("TRN2", target_bir_lowering=False, debug=False)

    xT = nc.dram_tensor("xT", [E, ROWS], F32R, kind="ExternalInput").ap()
    wq = nc.dram_tensor("wq", [E, HPC * D], F32R, kind="ExternalInput").ap()
    wk = nc.dram_tensor("wk", [E, HPC * D], F32R, kind="ExternalInput").ap()
    wv = nc.dram_tensor("wv", [E, HPC * D], F32R, kind="ExternalInput").ap()
    wp = nc.dram_tensor("wp", [HPC * D, E], F32R, kind="ExternalInput").ap()
    cos2 = nc.dram_tensor("cos2", [128, s_], F32, kind="ExternalInput").ap()
    sin2 = nc.dram_tensor("sin2", [128, s_], F32, kind="ExternalInput").ap()
    jmat = nc.dram_tensor("jmat", [128, 128], F32R, kind="ExternalInput").ap()
    trimask = nc.dram_tensor("trimask", [128, 128], F32R, kind="ExternalInput").ap()
    ident = nc.dram_tensor("ident", [128, 128], F32R, kind="ExternalInput").ap()
    onesd = nc.dram_tensor("onesd", [128, 128], F32R, kind="ExternalInput").ap()
    zerod = nc.dram_tensor("zerod", [128, 384], F32R, kind="ExternalInput").ap()
    out = nc.dram_tensor("out", [ROWS, E], F32, kind="ExternalOutput").ap()

    LN_SCALE = 1.0 / D
    LN_BIAS = QK_NORM_EPS
    EXP_SCALE = -0.5
    EXP_BIAS = -0.25 * float(np.log(D))  # folds 1/sqrt(D) into the q,k scales

    with tile.TileContext(nc) as tc, ExitStack() as ctx:
        wpool = ctx.enter_context(tc.tile_pool(name="weights", bufs=1))
        const = ctx.enter_context(tc.tile_pool(name="const", bufs=1))
        xtp = ctx.enter_context(tc.tile_pool(name="xt", bufs=2))
        qkv = ctx.enter_context(tc.tile_pool(name="qkv", bufs=1))
        tmp = ctx.enter_context(tc.tile_pool(name="tmp", bufs=2))
        expp = ctx.enter_context(tc.tile_pool(name="expp", bufs=2))
        ctxp = ctx.enter_context(tc.tile_pool(name="ctxp", bufs=1))
        outp = ctx.enter_context(tc.tile_pool(name="outp", bufs=2))

        # resident weights: [128, NE, HPC*D] with contraction slice e on free dim
        wq_s = wpool.tile([128, NE, HPC * D], F32R, tag="wqs")
        wk_s = wpool.tile([128, NE, HPC * D], F32R, tag="wks")
        wv_s = wpool.tile([128, NE, HPC * D], F32R, tag="wvs")
        wp_s = wpool.tile([128, HPC, E], F32R, tag="wps")
        nc.sync.dma_start(out=wq_s, in_=wq.rearrange("(ne p) m -> p ne m", p=128))
        nc.sync.dma_start(out=wk_s, in_=wk.rearrange("(ne p) m -> p ne m", p=128))
        nc.sync.dma_start(out=wv_s, in_=wv.rearrange("(ne p) m -> p ne m", p=128))
        nc.sync.dma_start(out=wp_s, in_=wp.rearrange("(h p) m -> p h m", p=128))

        cos_s = const.tile([128, s_], F32, tag="cos")
        sin_s = const.tile([128, s_], F32, tag="sin")
        j_s = const.tile([128, 128], F32R, tag="jmat")
        tri_s = const.tile([128, 128], F32R, tag="tri")
        id_s = const.tile([128, 128], F32R, tag="ident")
        ones_s = const.tile([128, 128], F32R, tag="ones")
        zero_s = const.tile([128, 384], F32R, tag="zeros")
        nc.sync.dma_start(out=ones_s, in_=onesd)
        nc.sync.dma_start(out=zero_s, in_=zerod)
        bias_ln = const.tile([128, 1], F32, tag="bias_ln")
        bias_ex = const.tile([128, 1], F32, tag="bias_ex")
        nc.vector.memset(bias_ln, LN_BIAS)
        nc.vector.memset(bias_ex, EXP_BIAS)
        nc.sync.dma_start(out=cos_s, in_=cos2)
        nc.sync.dma_start(out=sin_s, in_=sin2)
        nc.sync.dma_start(out=j_s, in_=jmat)
        nc.sync.dma_start(out=tri_s, in_=trimask)
        nc.sync.dma_start(out=id_s, in_=ident)

        for b in range(b_):
            # ---------- phase A: QKV projection + V transpose + rope + norm
            qtn = [qkv.tile([128, s_], F32R, tag=f"qtn{h}", name=f"qtn{h}") for h in range(HPC)]
            ktn = [qkv.tile([128, s_], F32R, tag=f"ktn{h}", name=f"ktn{h}") for h in range(HPC)]
            vsb = [qkv.tile([128, s_ // 128, D], F32R, tag=f"vsb{h}", name=f"vsb{h}")
                   for h in range(HPC)]

            with tc.tile_pool(name=f"psA{b}", bufs=1, space="PSUM") as psA:
                for rc in range(s_ // RC):
                    col0 = b * s_ + rc * RC
                    csl = slice(rc * RC, rc * RC + RC)
                    p_q = [psA.tile([128, RC], F32, tag=f"pq{h}", name=f"pq{h}") for h in range(HPC)]
                    p_k = [psA.tile([128, RC], F32, tag=f"pk{h}", name=f"pk{h}") for h in range(HPC)]
                    p_vt = [psA.tile([128, RC], F32, tag=f"pvt{h}", name=f"pvt{h}") for h in range(HPC)]

                    for e in range(NE):
                        xt = xtp.tile([128, RC], F32R, tag="xt")
                        nc.sync.dma_start(
                            out=xt, in_=xT[e * 128:(e + 1) * 128, col0:col0 + RC])
                        st, sp = (e == 0), (e == NE - 1)
                        for h in range(HPC):
                            hsl = slice(h * D, (h + 1) * D)
                            nc.tensor.matmul(p_q[h], wq_s[:, e, hsl], xt,
                                             start=st, stop=sp)
                            nc.tensor.matmul(p_k[h], wk_s[:, e, hsl], xt,
                                             start=st, stop=sp)
                            nc.tensor.matmul(p_vt[h], wv_s[:, e, hsl], xt,
                                             start=st, stop=sp)

                    # V: evacuate VT, then PE-transpose 128-blocks into natural V
                    for h in range(HPC):
                        vt_sb = tmp.tile([128, RC], F32R, tag="vt")
                        nc.vector.tensor_copy(vt_sb, p_vt[h])
                        for pt in range(RC // 128):
                            kt = (rc * RC) // 128 + pt
                            p_tr = psA.tile([128, 128], F32R, tag="scratch")
                            nc.tensor.transpose(
                                p_tr, vt_sb[:, pt * 128:(pt + 1) * 128], id_s)
                            nc.vector.tensor_copy(vsb[h][:, kt, :], p_tr)

                    # Q/K: rmsnorm scale via ones-matmul + Ln/Exp, rope via J
                    for h in range(HPC):
                        for psrc, dst in ((p_q[h], qtn[h]), (p_k[h], ktn[h])):
                            raw = tmp.tile([128, RC], F32R, tag="raw")
                            nc.vector.tensor_copy(raw, psrc)
                            sq = tmp.tile([128, RC], F32R, tag="sq")
                            nc.vector.tensor_mul(sq, raw, raw)
                            p_ss = psA.tile([128, RC], F32, tag="scratch")
                            nc.tensor.matmul(p_ss, ones_s, sq, start=True, stop=True)
                            lnt = tmp.tile([128, RC], F32, tag="t1")
                            nc.scalar.activation(lnt, p_ss, AF.Ln,
                                                 scale=LN_SCALE, bias=bias_ln)
                            rq = tmp.tile([128, RC], F32, tag="sq")
                            nc.scalar.activation(rq, lnt, AF.Exp,
                                                 scale=EXP_SCALE, bias=bias_ex)
                            p_jq = psA.tile([128, RC], F32, tag="scratch")
                            nc.tensor.matmul(p_jq, j_s, raw, start=True, stop=True)
                            t1 = tmp.tile([128, RC], F32, tag="t1")
                            nc.vector.tensor_mul(t1, raw, cos_s[:, csl])
                            t2 = tmp.tile([128, RC], F32, tag="t2")
                            nc.vector.tensor_mul(t2, p_jq, sin_s[:, csl])
                            t3 = tmp.tile([128, RC], F32, tag="raw")
                            nc.vector.tensor_add(t3, t1, t2)
                            nc.vector.tensor_mul(dst[:, csl], t3, rq)

            # ---------- phase B: causal attention ------------------------
            ctxTs = [ctxp.tile([128, s_], F32R, tag=f"ctxT{h}", name=f"ctxT{h}") for h in range(HPC)]
            with tc.tile_pool(name=f"psB{b}", bufs=1, space="PSUM") as psB:
                for h in range(HPC):
                    for qb in range(NQB):
                        qsl = slice(qb * QB, (qb + 1) * QB)
                        p_ctx = psB.tile([128, QB], F32, tag="p_ctx")
                        p_rs = psB.tile([128, QB], F32, tag="p_rs")
                        n_kt = (qb + 1) * KPQ
                        for g in range(max(1, n_kt // 2)):
                            kts = [kt for kt in (2 * g, 2 * g + 1) if kt < n_kt]
                            p_s = psB.tile([128, 2 * QB], F32, tag="p_s")
                            for i, kt in enumerate(kts):
                                nc.tensor.matmul(
                                    p_s[:, i * QB:(i + 1) * QB],
                                    ktn[h][:, kt * 128:(kt + 1) * 128],
                                    qtn[h][:, qsl], start=True, stop=True)
                            ex = expp.tile([128, 2 * QB], F32R, tag="ex")
                            rels = [kt - qb * KPQ for kt in kts]
                            if all(r < 0 for r in rels):
                                nc.scalar.activation(ex[:, :len(kts) * QB],
                                                     p_s[:, :len(kts) * QB], AF.Exp)
                            else:
                                for i, kt in enumerate(kts):
                                    rel = rels[i]
                                    esl = ex[:, i * QB:(i + 1) * QB]
                                    psl = p_s[:, i * QB:(i + 1) * QB]
                                    if rel < 0:
                                        nc.scalar.activation(esl, psl, AF.Exp)
                                        continue
                                    if rel > 0:
                                        nc.vector.tensor_copy(
                                            esl[:, :rel * 128], zero_s[:, :rel * 128])
                                    nc.scalar.activation(
                                        esl[:, rel * 128:], psl[:, rel * 128:], AF.Exp)
                                    nc.vector.tensor_mul(
                                        esl[:, rel * 128:(rel + 1) * 128],
                                        esl[:, rel * 128:(rel + 1) * 128], tri_s)
                            for i, kt in enumerate(kts):
                                st, sp = (kt == 0), (kt == n_kt - 1)
                                nc.tensor.matmul(p_ctx, vsb[h][:, kt, :],
                                                 ex[:, i * QB:(i + 1) * QB],
                                                 start=st, stop=sp)
                                nc.tensor.matmul(p_rs, ones_s,
                                                 ex[:, i * QB:(i + 1) * QB],
                                                 start=st, stop=sp)
                        # 1/rowsum via Exp(-Ln(x)), broadcast over partitions
                        lnr = tmp.tile([128, QB], F32, tag="lnr")
                        nc.scalar.activation(lnr, p_rs, AF.Ln)
                        rs = tmp.tile([128, QB], F32, tag="rs")
                        nc.scalar.activation(rs, lnr, AF.Exp, scale=-1.0)
                        nc.vector.tensor_mul(ctxTs[h][:, qsl], p_ctx, rs)

            # ---------- phase C: output projection -----------------------
            with tc.tile_pool(name=f"psC{b}", bufs=2, space="PSUM") as psC:
                for rt in range(s_ // 128):
                    rsl = slice(rt * 128, (rt + 1) * 128)
                    p_o = psC.tile([128, E], F32, tag="po")
                    for h in range(HPC):
                        for nch in range(E // 512):
                            nc.tensor.matmul(
                                p_o[:, nch * 512:(nch + 1) * 512],
                                ctxTs[h][:, rsl],
                                wp_s[:, h, nch * 512:(nch + 1) * 512],
                                start=(h == 0), stop=(h == HPC - 1))
                    o_sb = outp.tile([128, E], F32, tag="o_sb")
                    nc.vector.tensor_copy(o_sb[:, :E // 2], p_o[:, :E // 2])
                    nc.scalar.activation(o_sb[:, E // 2:], p_o[:, E // 2:], AF.Copy)
                    nc.sync.dma_start(
                        out=out[b * s_ + rt * 128: b * s_ + (rt + 1) * 128, :],
                        in_=o_sb)

    nc.compile()
    return nc


def host_inputs(x, w_qkv, w_proj, core, s_=None):
    """Per-core input map (numpy, all f32)."""
    b_, s_x, e = x.shape
    s_ = s_x if s_ is None else s_
    xT = np.ascontiguousarray(x.reshape(b_ * s_, e).T)

    hs = [core * HPC + i for i in range(HPC)]
    perm = np.concatenate([np.arange(0, D, 2), np.arange(1, D, 2)])
    wq_c = np.concatenate(
        [w_qkv[:, 0 * e + h * D:0 * e + (h + 1) * D][:, perm] for h in hs], axis=1)
    wk_c = np.concatenate(
        [w_qkv[:, 1 * e + h * D:1 * e + (h + 1) * D][:, perm] for h in hs], axis=1)
    wv_c = np.concatenate(
        [w_qkv[:, 2 * e + h * D:2 * e + (h + 1) * D] for h in hs], axis=1)
    wp_c = np.concatenate([w_proj[h * D:(h + 1) * D, :] for h in hs], axis=0)

    inv_freq = 1.0 / (ROPE_BASE ** (np.arange(0, D, 2, dtype=np.float64) / D))
    t = np.arange(s_, dtype=np.float64)
    freqs = np.outer(inv_freq, t)            # [64, S]
    cosT = np.cos(freqs).astype(np.float32)
    sinT = np.sin(freqs).astype(np.float32)
    cos2 = np.vstack([cosT, cosT])
    sin2 = np.vstack([sinT, sinT])

    J = np.zeros((128, 128), np.float32)
    for r in range(64):
        J[r, r + 64] = -1.0
        J[r + 64, r] = 1.0
    jmat = np.ascontiguousarray(J.T)

    ki, qi = np.meshgrid(np.arange(128), np.arange(128), indexing="ij")
    trimask = (ki <= qi).astype(np.float32)
    ident = np.eye(128, dtype=np.float32)

    return {
        "xT": xT, "wq": np.ascontiguousarray(wq_c),
        "wk": np.ascontiguousarray(wk_c), "wv": np.ascontiguousarray(wv_c),
        "wp": np.ascontiguousarray(wp_c), "cos2": cos2, "sin2": sin2,
        "jmat": jmat, "trimask": trimask, "ident": ident,
        "onesd": np.ones((128, 128), np.float32),
        "zerod": np.zeros((128, 384), np.float32),
    }


_CACHE = {}


def _get_nc(b_, s_):
    key = (b_, s_)
    if key not in _CACHE:
        _CACHE[key] = build_kernel(b_, s_)
    return _CACHE[key]


def kernel(x, w_qkv, w_proj):
    x = np.asarray(x, dtype=np.float32)
    w_qkv = np.asarray(w_qkv, dtype=np.float32)
    w_proj = np.asarray(w_proj, dtype=np.float32)
    b_, s_, e = x.shape

    nc = _get_nc(b_, s_)
    in_maps = [host_inputs(x, w_qkv, w_proj, c) for c in range(N_CORES)]
    res = run_bass_kernel_spmd(nc, in_maps, list(range(N_CORES)))
    acc = res.results[0]["out"].astype(np.float32).copy()
    for c in range(1, N_CORES):
        acc += res.results[c]["out"]
    return acc.reshape(b_, s_, e)
